# revision 31
# baseline (speedup 1.0000x reference)
"""Distributed Trainium2 Bass kernel for BNHC hypergraph encoder (8 NeuronCores).

Strategy:
  - Host: degree-norm coefficients; incidences double-sharded (phase A by
    hyperedge block, phase B by node block); per destination-supertile (64
    rows) token groups, each split lo/hi for the int16 gather-index limit and
    padded to core-uniform lengths (SPMD: one graph, 8 cores).
  - Device per layer: xt = h @ W + b (TensorE); bf16 xt table AllGather;
    phase A: dma_gather(bf16 rows) + DVE-built coef-onehot tiles
    (iota==seg)*coef -> TensorE segment-sum into PSUM per supertile ->
    SBUF stage; cast bf16 -> AllGather hyper table; phase B symmetric ->
    z stage in SBUF; h = relu(h + z). No DRAM scatter (dma_scatter_add
    races on duplicate destinations), no staging round-trips.
  - Output: per-core h slice, concatenated on host.
"""
import os
import sys

sys.path.insert(0, "/opt/trn_rl_repo")
import numpy as np
import concourse.bass as bass
import concourse.mybir as mybir

F32 = mybir.dt.float32
BF16 = mybir.dt.bfloat16
I16 = mybir.dt.int16

CORES = 8
N = 50000
E = 50000
M = 640000
IN_DIM = 256
D = 128
NLAYERS = 3
CHUNK = 512  # tokens per gather chunk (>512-idx dma_gather calls hang on HW)
STW = 64      # supertile width (dst rows per PSUM accumulation group)


def _ceil(a, b):
    return (a + b - 1) // b


def _pad128(n):
    return _ceil(max(n, 1), 128) * 128


def wrap_idx(idx):
    """int16 token index layout: token i -> partition i%16, col i//16, x8 replicated."""
    Lt = len(idx)
    arr = idx.reshape(Lt // 16, 16).T
    return np.ascontiguousarray(np.tile(arr, (8, 1)))


def coef_layout(cf):
    """f32 layout matching gather output: token i -> partition i%128, col i//128."""
    Lt = len(cf)
    return np.ascontiguousarray(cf.reshape(Lt // 128, 128).T)


def _phase_lists(gsrc, sdst, coef, owner, S, n_cores, lo_limit):
    """Segment-sum layout: per-core (gidx, coef, segl) streams ordered
    [st0_lo..stN_lo, st0_hi..stN_hi], each block core-uniformly padded."""
    ST = _ceil(S, STW)
    buckets = []
    for c in range(n_cores):
        sel = np.nonzero(owner == c)[0]
        g = gsrc[sel]
        dl = (sdst[sel] - c * S).astype(np.int64)
        cf = coef[sel]
        st = dl // STW
        segl = (dl - st * STW).astype(np.float32)
        per_st = []
        for s in range(ST):
            m = st == s
            gs, cs, ss = g[m], cf[m], segl[m]
            lo = gs < lo_limit
            per_st.append(((gs[lo], cs[lo], ss[lo]),
                           (gs[~lo] - lo_limit, cs[~lo], ss[~lo])))
        buckets.append(per_st)
    lo_lens = [_pad128(max(len(buckets[c][s][0][0]) for c in range(n_cores)))
               for s in range(ST)]
    hi_lens = [_pad128(max(len(buckets[c][s][1][0]) for c in range(n_cores)))
               for s in range(ST)]
    total = sum(lo_lens) + sum(hi_lens)
    out = []
    for c in range(n_cores):
        gi = np.zeros(total, np.int16)
        cf = np.zeros(total, np.float32)
        sg = np.zeros(total, np.float32)
        off = 0
        for part, lens in ((0, lo_lens), (1, hi_lens)):
            for s in range(ST):
                gs, cs, ss = buckets[c][s][part]
                gi[off:off + len(gs)] = gs
                cf[off:off + len(cs)] = cs
                sg[off:off + len(ss)] = ss
                off += lens[s]
        out.append((wrap_idx(gi), coef_layout(cf), coef_layout(sg)))
    return out, lo_lens, hi_lens


def preprocess(x, hyperedge_index, weightMatrix, edgesWeights, n, e, n_cores):
    row = hyperedge_index[0].astype(np.int64)
    col = hyperedge_index[1].astype(np.int64)
    wM = weightMatrix.astype(np.float32)
    eW = edgesWeights.astype(np.float32)
    node_deg = np.bincount(row, weights=wM[col], minlength=n).astype(np.float32) + 1e-8
    hyper_deg = np.bincount(col, weights=eW, minlength=e).astype(np.float32) + 1e-8
    nd = node_deg ** -0.5
    nd[np.isinf(nd)] = 0.0
    hd = hyper_deg ** -0.5
    hd[np.isinf(hd)] = 0.0
    norm = (hd[col] * nd[row]).astype(np.float32)
    coef1 = (eW * norm).astype(np.float32)
    coef2 = (wM[col] * eW * norm).astype(np.float32)
    S = n // n_cores
    lo_limit = 32768 if n > 32768 else max(128, _pad128(n // 2))
    a_lists, a_lo, a_hi = _phase_lists(row, col, coef1, col // S, S, n_cores, lo_limit)
    b_lists, b_lo, b_hi = _phase_lists(col, row, coef2, row // S, S, n_cores, lo_limit)
    return a_lists, a_lo, a_hi, b_lists, b_lo, b_hi, lo_limit


def _tile_schedule(lo_lens, hi_lens, chunk):
    """(tiles, chunks): tiles[t] = (session, first, stop, is_hi); chunks =
    [(start_tok, ntok, is_hi)]. Sessions: lo supertiles 0..ST-1 then hi ST..2ST-1."""
    ST = len(lo_lens)
    tiles = []
    for part, lens in ((0, lo_lens), (1, hi_lens)):
        for s in range(ST):
            nt = lens[s] // 128
            for k in range(nt):
                tiles.append((part * ST + s, k == 0, k == nt - 1, part == 1))
    lo_total, hi_total = sum(lo_lens), sum(hi_lens)
    chunks = []
    for base, ln, is_hi in ((0, lo_total, False), (lo_total, hi_total, True)):
        s = 0
        while s < ln:
            t = min(chunk, ln - s)
            chunks.append((base + s, t, is_hi))
            s += t
    return tiles, chunks


class Plan:
    """Per-engine op lists; python-tracked absolute semaphore thresholds."""

    DMA_SEMS = ("dmaI", "dmaX0", "dmaX1", "dmaS", "gs0", "gs1")

    def __init__(self, engines, sems):
        self.ops = {e: [] for e in engines}
        self.count = {k: 0 for k in sems}
        self.sems = sems

    def add(self, engine, emit, waits=(), inc=None, inc_by=1):
        self.ops[engine].append((list(waits), emit, inc, inc_by))
        if inc is not None:
            self.count[inc] += inc_by * (16 if inc in self.DMA_SEMS else 1)

    def run(self, engine, eng):
        for waits, emit, inc, inc_by in self.ops[engine]:
            for s, thr in waits:
                if thr > 0:
                    eng.wait_ge(self.sems[s], thr)
            ins = emit(eng)
            if inc is not None and ins is not None:
                ins.then_inc(self.sems[inc], 16 if inc in self.DMA_SEMS else 1)


def build_graph(cfg):
    n, e, n_cores = cfg["N"], cfg["E"], cfg["CORES"]
    in_dim, d = cfg["IN_DIM"], cfg["D"]
    n_layers = cfg["L"]
    S = n // n_cores
    NBLK = _ceil(S, 128)
    LAST = S - (NBLK - 1) * 128
    a_lo_lens, a_hi_lens = cfg["A_LENS"]
    b_lo_lens, b_hi_lens = cfg["B_LENS"]
    LA = sum(a_lo_lens) + sum(a_hi_lens)
    LB = sum(b_lo_lens) + sum(b_hi_lens)
    CH = cfg["CHUNK"]
    GB = CH // 128
    KH = in_dim // 128
    lo_limit = cfg["LO_LIMIT"]
    SP = NBLK * 128
    ST = _ceil(S, STW)

    from concourse import bacc
    nc = bacc.Bacc(None, target_bir_lowering=False, debug=False,
                   detect_race_conditions=cfg.get("RACE_DETECT", False))

    xT = nc.declare_dram_parameter("xT", [KH, 128, SP], F32, isOutput=False)
    giA = nc.declare_dram_parameter("giA", [128, LA // 16], I16, isOutput=False)
    cfA = nc.declare_dram_parameter("cfA", [128, LA // 128], F32, isOutput=False)
    sgA = nc.declare_dram_parameter("sgA", [128, LA // 128], F32, isOutput=False)
    giB = nc.declare_dram_parameter("giB", [128, LB // 16], I16, isOutput=False)
    cfB = nc.declare_dram_parameter("cfB", [128, LB // 128], F32, isOutput=False)
    sgB = nc.declare_dram_parameter("sgB", [128, LB // 128], F32, isOutput=False)
    fc_w = nc.declare_dram_parameter("fc_w", [KH, 128, d], F32, isOutput=False)
    fc_b = nc.declare_dram_parameter("fc_b", [128, d], F32, isOutput=False)
    conv_w = nc.declare_dram_parameter("conv_w", [n_layers, d, d], F32, isOutput=False)
    conv_b = nc.declare_dram_parameter("conv_b", [128, n_layers, d], F32, isOutput=False)
    ident = nc.declare_dram_parameter("ident", [128, 128], F32, isOutput=False)
    iota = nc.declare_dram_parameter("iota", [128, GB, STW], F32, isOutput=False)
    out_ext = nc.declare_dram_parameter("out", [S, d], F32, isOutput=True)

    xt_bounce = nc.dram_tensor("xt_bounce", [S, d], BF16)
    xt_tab = nc.dram_tensor("xt_tab", [n, d], BF16, addr_space="Shared")
    hy_bounce = nc.dram_tensor("hy_bounce", [S, d], BF16)
    hy_tab = nc.dram_tensor("hy_tab", [e, d], BF16, addr_space="Shared")

    GP, SY, VE, TE, SC = "gpsimd", "sync", "vector", "tensor", "scalar"
    rg = [list(range(n_cores))]

    import contextlib
    with contextlib.ExitStack() as ctx:
        def sb(name, shape, dt):
            return ctx.enter_context(nc.sbuf_tensor(name, shape, dt))

        def ps(name, shape, dt):
            return ctx.enter_context(nc.psum_tensor(name, shape, dt))

        h_s = sb("h_s", [128, NBLK, d], F32)
        stage_f = sb("stage_f", [128, NBLK, d], F32)
        stage_bf = sb("stage_bf", [128, NBLK, d], BF16)
        xtbuf = sb("xtbuf", [128, NBLK, d], BF16)
        gb_bf = [sb(f"gb_bf{i}", [128, GB, d], BF16) for i in range(2)]
        ch_bf = [sb(f"ch_bf{i}", [128, GB, STW], BF16) for i in range(2)]
        giA_s = sb("giA_s", [128, LA // 16], I16)
        cfA_s = sb("cfA_s", [128, LA // 128], F32)
        sgA_s = sb("sgA_s", [128, LA // 128], F32)
        giB_s = sb("giB_s", [128, LB // 16], I16)
        cfB_s = sb("cfB_s", [128, LB // 128], F32)
        sgB_s = sb("sgB_s", [128, LB // 128], F32)
        xts = [sb(f"xts{i}", [128, KH, 128], F32) for i in range(2)]
        fcw_s = sb("fcw_s", [128, KH, d], F32)
        fcb_s = sb("fcb_s", [128, d], F32)
        cvw_s = sb("cvw_s", [128, n_layers, d], F32)
        cvb_s = sb("cvb_s", [128, n_layers, d], F32)
        id_s = sb("id_s", [128, 128], F32)
        io_s = sb("io_s", [128, GB, STW], F32)
        hT_s = [sb(f"hT_s{i}", [128, 128], F32) for i in range(2)]
        ptT = [ps(f"ptT{i}", [128, 128], F32) for i in range(2)]
        ptM = [ps(f"ptM{i}", [128, 128], F32) for i in range(2)]
        psA = [ps(f"psA{i}", [STW, d], F32) for i in range(2)]

        sem_names = ["dmaI", "dmaX0", "dmaX1", "dmaS", "gs0", "gs1",
                     "vs", "ts", "cs", "cc", "ms"]
        sems = {nm: ctx.enter_context(nc.semaphore(nm)) for nm in sem_names}
        block = ctx.enter_context(nc.Block())

        plan = Plan([GP, SY, VE, TE, SC], sems)
        C = plan.count

        def dma(dst, src):
            return lambda g: g.dma_start(out=dst, in_=src)

        # ---- init ----
        for dst, src in [
            (giA_s[:], giA[:, :]), (cfA_s[:], cfA[:, :]), (sgA_s[:], sgA[:, :]),
            (giB_s[:], giB[:, :]), (cfB_s[:], cfB[:, :]), (sgB_s[:], sgB[:, :]),
            (fcw_s[:], fc_w[:, :, :].transpose([1, 0, 2])), (fcb_s[:], fc_b[:, :]),
            (cvw_s[:], conv_w[:, :, :].transpose([1, 0, 2])),
            (cvb_s[:], conv_b[:, :, :]),
            (id_s[:], ident[:, :]), (io_s[:], iota[:, :, :]),
        ]:
            plan.add(SY, dma(dst, src), inc="dmaI")
        init_dma = C["dmaI"]

        for t in (*gb_bf, stage_f):
            plan.add(GP, (lambda tt: lambda g: g.memset(tt[:], 0.0))(t), inc="ms")
        n_memset = C["ms"]

        # ---- fc layer ----
        for b in range(NBLK):
            buf = b % 2
            lane = f"dmaX{buf}"
            plan.add(SY, dma(xts[buf][:], xT[:, :, b * 128:(b + 1) * 128].transpose([1, 0, 2])),
                     waits=([("ts", C["ts"] - 1)] if b >= 2 else []), inc=lane)
            for k in range(KH):
                plan.add(TE, (lambda kk, bf: lambda t: t.matmul(
                    ptM[bf][:], xts[bf][:, kk, :], fcw_s[:, kk, :],
                    start=(kk == 0), stop=(kk == KH - 1)))(k, buf),
                    waits=[(lane, C[lane]), ("dmaI", init_dma)]
                    + ([("vs", C["vs"] - 1)] if b >= 2 and k == 0 else []),
                    inc=("ts" if k == KH - 1 else None))

            def brelu(bb, bf):
                def f(v):
                    v.tensor_tensor(out=h_s[:, bb, :], in0=ptM[bf][:],
                                    in1=fcb_s[:, :], op=mybir.AluOpType.add)
                    return v.tensor_relu(out=h_s[:, bb, :], in_=h_s[:, bb, :])
                return f
            plan.add(VE, brelu(b, buf), waits=[("ts", C["ts"])], inc="vs")

        def rect_mover(to_sbuf, tile, stage_t, b0, nb, rows, lane="dmaS"):
            nfull = rows // 128
            rag = rows - nfull * 128
            def f(g):
                last = None
                if nfull:
                    dram = stage_t[b0 * 128:(b0 + nfull) * 128, :].rearrange(
                        "(b p) e -> p b e", p=128)
                    if to_sbuf:
                        last = g.dma_start(out=tile[:, :nfull, :], in_=dram)
                    else:
                        last = g.dma_start(out=dram, in_=tile[:, :nfull, :])
                    if rag:
                        last.then_inc(sems[lane], 16)
                if rag:
                    if to_sbuf:
                        last = g.dma_start(out=tile[:rag, nfull, :],
                                           in_=stage_t[(b0 + nfull) * 128:b0 * 128 + rows, :])
                    else:
                        last = g.dma_start(out=stage_t[(b0 + nfull) * 128:b0 * 128 + rows, :],
                                           in_=tile[:rag, nfull, :])
                return last
            return f, (1 if rag == 0 or nfull == 0 else 2)

        def xt_compute(li):
            h_ready = C["vs"]
            for b in range(NBLK):
                buf = b % 2
                plan.add(TE, (lambda bb, bf: lambda t: t.transpose(
                    ptT[bf][:], h_s[:, bb, :], id_s[:]))(b, buf),
                    waits=[("vs", h_ready)] + ([("cs", C["cs"] - 1)] if b >= 2 else []),
                    inc="ts")
                plan.add(SC, (lambda bf: lambda s: s.copy(hT_s[bf][:], ptT[bf][:]))(buf),
                         waits=[("ts", C["ts"])], inc="cs")
                plan.add(TE, (lambda bf: lambda t: t.matmul(
                    ptM[bf][:], hT_s[bf][:], cvw_s[:, li, :], start=True, stop=True))(buf),
                    waits=[("cs", C["cs"])] + ([("vs", C["vs"] - 1)] if b >= 2 else []),
                    inc="ts")
                plan.add(VE, (lambda bb, bf: lambda v: v.tensor_tensor(
                    out=xtbuf[:, bb, :], in0=ptM[bf][:],
                    in1=cvb_s[:, li, :], op=mybir.AluOpType.add))(b, buf),
                    waits=[("ts", C["ts"])], inc="vs")
            emit, nd = rect_mover(False, xtbuf, xt_bounce, 0, NBLK, S)
            plan.add(SY, emit, waits=[("vs", C["vs"])], inc="dmaS", inc_by=nd)
            plan.add(GP, lambda g: g.collective_compute(
                "AllGather", mybir.AluOpType.bypass, replica_groups=rg,
                ins=[xt_bounce.ap().opt()], outs=[xt_tab.ap().opt()]),
                waits=[("dmaS", C["dmaS"])], inc="cc")

        gp_regs = {}
        reg_vals = set()

        def phase(tab, gi_s, cf_s, sg_s, lo_lens, hi_lens, gate_cc, stage_free_vs):
            """Segment-sum phase. stage_free_vs: vs count after which stage_f
            may be overwritten (prev consumer of stage_f done)."""
            tiles, chunks = _tile_schedule(lo_lens, hi_lens, CH)
            tile_of = []
            for ci, (start, ntok, _) in enumerate(chunks):
                for k in range(ntok // 128):
                    tile_of.append((ci, k))
            assert len(tile_of) == len(tiles)
            ts_at_phase_start = C["ts"]
            vs_at_phase_start = C["vs"]
            cs_at_phase_start = C["cs"]
            chunk_fence = {}      # ci -> ts after last matmul of chunk
            sess_fence = {}       # session q -> ("cs"|"vs", count) consumer fence
            gather_done = {}      # ci -> gs count
            ch_done = {}          # ci -> vs count
            # map global tile index -> last tile of its chunk?
            last_tile_of_chunk = {}
            for t, (ci, k) in enumerate(tile_of):
                last_tile_of_chunk[ci] = t

            for ci, (start, ntok, is_hi) in enumerate(chunks):
                reg_vals.add(ntok)
                buf = ci % 2
                nb = ntok // 128
                src = tab[lo_limit:, :] if is_hi else tab[:, :]
                waits = []
                if ci == 0:
                    waits += [("cc", gate_cc), ("ms", n_memset)]
                if ci < 2:
                    waits += [("ts", ts_at_phase_start)]
                else:
                    waits += [("ts", chunk_fence[ci - 2])]
                glane = f"gs{buf}"
                plan.add(GP, (lambda sr, st, nt, bf, nbk: lambda g: g.dma_gather(
                    gb_bf[bf][:, :nbk, :], sr, gi_s[:, st // 16:(st + nt) // 16],
                    num_idxs=nt, num_idxs_reg=gp_regs[nt], elem_size=d))(src, start, ntok, buf, nb),
                    waits=waits, inc=glane)
                gather_done[ci] = (glane, C[glane])
                # coefhot build: eq then scale (batched over chunk tiles)
                t0 = start // 128
                vw = ([("ts", chunk_fence[ci - 2])] if ci >= 2
                      else [("dmaI", init_dma), ("ts", ts_at_phase_start)])
                plan.add(VE, (lambda tt0, bf, nbk: lambda v: v.tensor_tensor(
                    out=ch_bf[bf][:, :nbk, :], in0=io_s[:, :nbk, :],
                    in1=sg_s[:, tt0:tt0 + nbk].broadcast_to([128, nbk, STW]),
                    op=mybir.AluOpType.is_equal))(t0, buf, nb),
                    waits=vw, inc="vs")
                plan.add(VE, (lambda tt0, bf, nbk: lambda v: v.tensor_tensor(
                    out=ch_bf[bf][:, :nbk, :], in0=ch_bf[bf][:, :nbk, :],
                    in1=cf_s[:, tt0:tt0 + nbk].broadcast_to([128, nbk, STW]),
                    op=mybir.AluOpType.mult))(t0, buf, nb),
                    inc="vs")
                ch_done[ci] = C["vs"]

                # matmuls for this chunk's tiles
                for k in range(nb):
                    t = sum(c[1] // 128 for c in chunks[:ci]) + k
                    q, first, stop, _ = tiles[t]
                    waits = []
                    if k == 0:
                        waits += [gather_done[ci], ("vs", ch_done[ci])]
                    if first:
                        if q >= 2 and (q - 2) in sess_fence:
                            sem_nm, cnt = sess_fence[q - 2]
                            waits += [(sem_nm, cnt)]
                        elif q < 2:
                            # psA reused from the previous phase's tail sessions
                            waits += [("vs", vs_at_phase_start),
                                      ("cs", cs_at_phase_start)]
                    plan.add(TE, (lambda qq, bf, kk, fr, st_: lambda te: te.matmul(
                        psA[qq % 2][:, :], ch_bf[bf][:, kk, :], gb_bf[bf][:, kk, :],
                        start=fr, stop=st_))(q, buf, k, first, stop),
                        waits=waits, inc="ts")
                    if stop:
                        s_act = q % ST
                        p0 = (s_act * STW) % 128
                        blk = (s_act * STW) // 128
                        if q < ST:  # lo pass: copy psum -> stage
                            cw = [("ts", C["ts"])]
                            if q < 2:
                                cw += [("vs", stage_free_vs), ("ms", n_memset)]
                            plan.add(SC, (lambda qq, pp0, bb: lambda s: s.copy(
                                stage_f[pp0:pp0 + STW, bb, :], psA[qq % 2][:, :]))(q, p0, blk),
                                waits=cw, inc="cs")
                            sess_fence[q] = ("cs", C["cs"])
                        else:       # hi pass: add psum into stage
                            lo_q = q - ST
                            aw = [("ts", C["ts"]), ("cs", sess_fence[lo_q][1])]
                            plan.add(VE, (lambda qq, pp0, bb: lambda v: v.tensor_tensor(
                                out=stage_f[pp0:pp0 + STW, bb, :],
                                in0=stage_f[pp0:pp0 + STW, bb, :],
                                in1=psA[qq % 2][:, :], op=mybir.AluOpType.add))(q, p0, blk),
                                waits=aw, inc="vs")
                            sess_fence[q] = ("vs", C["vs"])
                chunk_fence[ci] = C["ts"]

        for li in range(cfg["L"]):
            xt_compute(li)
            ag_xt = C["cc"]
            # stage_f free after previous layer's h-update (vs) or anytime for layer 0
            phase(xt_tab, giA_s, cfA_s, sgA_s, a_lo_lens, a_hi_lens, ag_xt, C["vs"])
            # cast + AG hyper table
            plan.add(VE, lambda v: v.tensor_copy(stage_bf[:], stage_f[:]),
                     waits=[("cs", C["cs"])], inc="vs")
            cast_done = C["vs"]
            emit, nd = rect_mover(False, stage_bf, hy_bounce, 0, NBLK, S)
            plan.add(SY, emit, waits=[("vs", cast_done)], inc="dmaS", inc_by=nd)
            plan.add(GP, lambda g: g.collective_compute(
                "AllGather", mybir.AluOpType.bypass, replica_groups=rg,
                ins=[hy_bounce.ap().opt()], outs=[hy_tab.ap().opt()]),
                waits=[("dmaS", C["dmaS"])], inc="cc")
            ag_hy = C["cc"]
            phase(hy_tab, giB_s, cfB_s, sgB_s, b_lo_lens, b_hi_lens, ag_hy, cast_done)

            # h = relu(h + z)
            def hupd(v):
                v.tensor_tensor(out=h_s[:], in0=h_s[:], in1=stage_f[:],
                                op=mybir.AluOpType.add)
                return v.tensor_relu(out=h_s[:], in_=h_s[:])
            plan.add(VE, hupd, waits=[("cs", C["cs"])], inc="vs")

        emit, nd = rect_mover(False, h_s, out_ext, 0, NBLK, S)
        plan.add(SY, emit, waits=[("vs", C["vs"])], inc="dmaS", inc_by=nd)
        final_dma = C["dmaS"]

        @block.gpsimd
        def _(g: bass.BassGpSimd):
            for v in sorted(reg_vals):
                r = g.alloc_register(f"ntok_{v}")
                g.reg_mov(r, v)
                gp_regs[v] = r
            plan.run(GP, g)

        @block.sync
        def _(s):
            plan.run(SY, s)
            s.wait_ge(sems["dmaS"], final_dma)

        @block.vector
        def _(v):
            plan.run(VE, v)

        @block.tensor
        def _(t):
            plan.run(TE, t)

        @block.scalar
        def _(s):
            plan.run(SC, s)

    nc.finalize()
    return nc


def make_in_maps(x, fc_w, fc_b, conv_w, conv_b, pre, cfg):
    a_lists, _, _, b_lists, _, _, _ = pre
    n, n_cores, in_dim, d = cfg["N"], cfg["CORES"], cfg["IN_DIM"], cfg["D"]
    n_layers = cfg["L"]
    S = n // n_cores
    NBLK = _ceil(S, 128)
    SP = NBLK * 128
    KH = in_dim // 128
    GB = cfg["CHUNK"] // 128
    fc_w2 = np.ascontiguousarray(fc_w.astype(np.float32).reshape(KH, 128, d))
    conv_w2 = np.ascontiguousarray(conv_w.astype(np.float32))
    conv_b2 = np.ascontiguousarray(np.broadcast_to(
        conv_b.astype(np.float32).reshape(1, n_layers, d), (128, n_layers, d)))
    ident = np.eye(128, dtype=np.float32)
    iota = np.ascontiguousarray(np.broadcast_to(
        np.arange(STW, dtype=np.float32)[None, None, :], (128, GB, STW)))
    fc_b2 = np.ascontiguousarray(np.broadcast_to(
        fc_b.astype(np.float32).reshape(1, d), (128, d)))
    in_maps = []
    for c in range(n_cores):
        xp = np.zeros((SP, in_dim), np.float32)
        xp[:S] = x[c * S:(c + 1) * S].astype(np.float32)
        xTc = np.ascontiguousarray(xp.T.reshape(KH, 128, SP))
        giA, cfA, sgA = a_lists[c]
        giB, cfB, sgB = b_lists[c]
        in_maps.append({
            "xT": xTc, "giA": giA, "cfA": cfA, "sgA": sgA,
            "giB": giB, "cfB": cfB, "sgB": sgB,
            "fc_w": fc_w2, "fc_b": fc_b2, "conv_w": conv_w2, "conv_b": conv_b2,
            "ident": ident, "iota": iota,
        })
    return in_maps


def run(x, hyperedge_index, weightMatrix, edgesWeights, fc_w, fc_b, conv_w, conv_b,
        cfg, backend="hw", trace=False):
    pre = preprocess(x, hyperedge_index, weightMatrix, edgesWeights,
                     cfg["N"], cfg["E"], cfg["CORES"])
    a_lists, a_lo, a_hi, b_lists, b_lo, b_hi, lo_limit = pre
    cfg = dict(cfg, A_LENS=(a_lo, a_hi), B_LENS=(b_lo, b_hi), LO_LIMIT=lo_limit)
    nc = build_graph(cfg)
    in_maps = make_in_maps(x, fc_w, fc_b, conv_w, conv_b, pre, cfg)
    n_cores = cfg["CORES"]
    if backend == "sim":
        from concourse import bass_interp
        sim = bass_interp.MultiCoreSim(nc, n_cores)
        for c in range(n_cores):
            for k, v in in_maps[c].items():
                sim.cores[c].tensor(k)[:] = v
        sim.simulate()
        outs = [np.array(sim.cores[c].mem_tensor("out")) for c in range(n_cores)]
        return np.concatenate(outs, 0), None
    from concourse.bass_utils import run_bass_kernel_spmd
    res = run_bass_kernel_spmd(nc, in_maps, list(range(n_cores)), trace=trace)
    outs = [res.results[c]["out"] for c in range(n_cores)]
    return np.concatenate(outs, 0), res


def kernel(x, hyperedge_index, weightMatrix, edgesWeights, fc_w, fc_b, conv_w, conv_b):
    cfg = {"N": N, "E": E, "CORES": CORES, "IN_DIM": IN_DIM, "D": D, "L": NLAYERS,
           "CHUNK": CHUNK}
    out, res = run(np.asarray(x), np.asarray(hyperedge_index), np.asarray(weightMatrix),
                   np.asarray(edgesWeights), np.asarray(fc_w), np.asarray(fc_b),
                   np.asarray(conv_w), np.asarray(conv_b), cfg, backend="hw")
    return out
